# revision 19
# baseline (speedup 1.0000x reference)
"""Trainium2 Bass kernel for nn_AtnScore (masked normalized-correlation softmax).

Math (per batch b):
  w = x2[b] viewed [C, N] (N = H*W, row-major), gram = w^T @ w  [N, N]
  a_l = 10 * (mask_l == 0) / max(||w[:,l]||, 1e-4)
  z[l, n] = a_l * gram[l, n]        (softmax over l, per column n)
  out[l, n] = max(softmax_l(z)[l, n] * (mask_l == 0), 1e-8)

Sharding: 8 cores = 4 batches x 2 column-halves (n in [0,2048) / [2048,4096)).
Each core computes z TRANSPOSED (partition = n-tile of its half, free = l) so
the softmax reduction runs along the free axis; the host gather transposes
back while upcasting.

Masked l columns have z identically 0 (a_l = 0): their softmax weight is
e^0 against a column max of >= 29.6, i.e. < 3e-10 of the sum, and their
outputs clamp to 1e-8. So the HOST packs only the unmasked l columns
(2020..2092 of 4096, padded with zero columns to NU=2176) — halving the
matmul, exp, normalize, and output-DMA work — and scatters the device
result into a 1e-8-filled canvas.

No max-reduce: the exp bias is a host-computed rigorous Cauchy-Schwarz
bound U0(n) = ||x16_n|| * max_l ||a_l x16_l|| boosted by +79; with E in
fp32 the whole column (worst observed slack 141 nats) fits fp32's ~175-nat
range. exp overflow is impossible by construction.

fp16 matmul operands run the PE at full rate (validated 3.2e-3 absmax
error vs the fp32 reference); the output is fp16.
"""

import numpy as np

B, C, HH, WW = 4, 256, 64, 64
N = HH * WW          # 4096 (l dimension, also total n)
NHALF = N // 2       # 2048 columns per core
P = 128              # partitions
KO = C // P          # 2 contraction tiles
NU = 2112            # packed unmasked-l capacity
CB = NU // 2         # 1056 per z tile (3 PSUM banks incl padding)
NT = NHALF // P      # 16 n-tiles per core
BOOST = 79.0

_CACHE = {}


def _build():
    import concourse.bacc as bacc
    import concourse.bass as bass
    import concourse.tile as tile
    import concourse.mybir as mybir
    from concourse.bass import ds

    f32 = mybir.dt.float32
    f16 = mybir.dt.float16
    Alu = mybir.AluOpType
    Act = mybir.ActivationFunctionType

    nc = bacc.Bacc(None, target_bir_lowering=False)

    x2s_d = nc.dram_tensor("x2s16", [C, NU], f16, kind="ExternalInput")
    x2n_d = nc.dram_tensor("x2n16", [C, NHALF], f16, kind="ExternalInput")
    nb_d = nc.dram_tensor("nbias", [P, NT], f32, kind="ExternalInput")
    out_d = nc.dram_tensor("out", [NHALF, NU], f16, kind="ExternalOutput")

    with tile.TileContext(nc) as tc:
        with tc.tile_pool(name="persist", bufs=1) as persist:
            x16s = persist.tile([P, KO, NU], f16)      # moving operand (packed)
            x16n = persist.tile([P, KO, NHALF], f16)   # stationary operand
            nbias = persist.tile([P, NT], f32)
            nc.sync.dma_start(nbias[:], nb_d[:])
            x2s_r = x2s_d[:].rearrange("(ko p) n -> p ko n", p=P)
            x2n_r = x2n_d[:].rearrange("(ko p) n -> p ko n", p=P)
            for ko in range(KO):
                nc.sync.dma_start(x16n[:, ko, :], x2n_r[:, ko, :])
                nc.sync.dma_start(x16s[:, ko, :], x2s_r[:, ko, :])

            with tc.tile_pool(name="zps", bufs=2, space="PSUM") as zps, \
                 tc.tile_pool(name="ebuf", bufs=3) as ebuf, \
                 tc.tile_pool(name="obuf", bufs=4) as obuf, \
                 tc.tile_pool(name="small", bufs=6) as small:
                # each z tile is CB=1088 wide: matmul sub-chunks 512+512+64
                subs = [(0, 512), (512, 512), (1024, 32)]
                for nt in range(NT):
                    E = ebuf.tile([P, NU], f32, name=f"E{nt}", tag="E")
                    ssum = small.tile([P, 2], f32, name=f"ssum{nt}", tag="ssum")
                    for zt in range(2):
                        z = zps.tile([P, CB], f32, name=f"z{nt}_{zt}", tag="z")
                        for ko in range(KO):
                            for off, w in subs:
                                nc.tensor.matmul(
                                    z[:, ds(off, w)],
                                    x16n[:, ko, ds(nt * P, P)],
                                    x16s[:, ko, ds(zt * CB + off, w)],
                                    start=(ko == 0), stop=(ko == KO - 1))
                        nc.scalar.activation(
                            E[:, ds(zt * CB, CB)], z[:], Act.Exp,
                            bias=nbias[:, ds(nt, 1)], scale=1.0,
                            accum_out=ssum[:, ds(zt, 1)])

                    stot = small.tile([P, 1], f32, name=f"st{nt}", tag="st")
                    nc.vector.reduce_sum(
                        stot[:], ssum[:], axis=mybir.AxisListType.X)
                    nc.vector.tensor_scalar_max(stot[:], stot[:], 1e-30)
                    rtot = small.tile([P, 1], f32, name=f"rt{nt}", tag="rt")
                    nc.vector.reciprocal_approx_fast(rtot[:], stot[:])

                    # normalize into the fp16 output staging tile, DMA out
                    o16 = obuf.tile([P, NU], f16, name=f"o{nt}", tag="o")
                    for zt in range(2):
                        nc.vector.tensor_scalar_mul(
                            o16[:, ds(zt * CB, CB)], E[:, ds(zt * CB, CB)],
                            rtot[:])
                    nc.gpsimd.dma_start(out_d[ds(nt * P, P), :], o16[:])
    nc.finalize()
    return nc


def _get_nc():
    if "nc" not in _CACHE:
        _CACHE["nc"] = _build()
    return _CACHE["nc"]


def _ensure_ntff_hook():
    """bass_utils under axon imports antenv.axon_hooks for trace=True; this
    image's antenv lacks it. Install a stub wired to the boot ctypes hook."""
    import sys
    import types
    try:
        import antenv.axon_hooks  # noqa: F401
        return
    except ImportError:
        pass
    mod = types.ModuleType("antenv.axon_hooks")
    _h = [None]
    mod.set_axon_ntff_profile_hook = lambda hook: _h.__setitem__(0, hook)
    mod.get_axon_ntff_profile_hook = lambda: _h[0]
    sys.modules["antenv.axon_hooks"] = mod
    try:
        import antenv
        antenv.axon_hooks = mod
    except ImportError:
        pass
    try:
        from trn_agent_boot.trn_boot import _ntff_profile_via_ctypes
        hook = _ntff_profile_via_ctypes("/opt/axon/libaxon_pjrt.so")
        if hook is not None:
            mod.set_axon_ntff_profile_hook(hook)
    except Exception:
        pass


def kernel(x2: np.ndarray, mask: np.ndarray) -> np.ndarray:
    from concourse.bass_utils import run_bass_kernel_spmd
    import os

    nc = _get_nc()
    x2 = np.ascontiguousarray(x2, dtype=np.float32)
    mask = np.ascontiguousarray(mask, dtype=np.float32)

    in_maps = []
    idxs = []
    for core in range(8):
        b, h = core // 2, core % 2
        xb = x2[b].reshape(C, N)
        mb = mask[b].reshape(N)
        idx = np.flatnonzero(mb == 0.0)
        assert len(idx) <= NU, f"unmasked count {len(idx)} exceeds NU={NU}"
        idxs.append(idx)
        sumsq = np.einsum("cn,cn->n", xb, xb, dtype=np.float64)
        norm = np.sqrt(sumsq).astype(np.float32)
        a = (10.0 / np.maximum(norm, 1e-4)).astype(np.float32)
        x2s16 = np.zeros((C, NU), dtype=np.float16)
        x2s16[:, :len(idx)] = (xb[:, idx] * a[None, idx]).astype(np.float16)
        x2n16 = np.ascontiguousarray(
            xb[:, h * NHALF:(h + 1) * NHALF]).astype(np.float16)
        # rigorous C-S bound on the f16 dot products, as the exp bias
        n16 = np.linalg.norm(x2n16.astype(np.float32), axis=0)
        y16max = float(np.linalg.norm(x2s16.astype(np.float32), axis=0).max())
        u0 = n16 * y16max * 1.001 + 0.5
        nbias = (BOOST - u0).astype(np.float32).reshape(NT, P).T  # [P, NT]
        in_maps.append({
            "x2s16": x2s16,
            "x2n16": x2n16,
            "nbias": np.ascontiguousarray(nbias),
        })

    trace = bool(int(os.environ.get("ATN_TRACE", "0")))
    if trace:
        _ensure_ntff_hook()
    res = run_bass_kernel_spmd(nc, in_maps, list(range(8)), trace=trace)
    if trace and res.exec_time_ns is not None:
        print(f"HW exec time: {res.exec_time_ns} ns")
        _CACHE["last_exec_ns"] = res.exec_time_ns
        _CACHE["last_results"] = res

    out = np.full((B, N, N), 1e-8, dtype=np.float32)
    for core in range(8):
        b, h = core // 2, core % 2
        idx = idxs[core]
        dev = res.results[core]["out"][:, :len(idx)].astype(np.float32).T
        np.maximum(dev, 1e-8, out=dev)
        out[b][idx, h * NHALF:(h + 1) * NHALF] = dev
    return out.reshape(B, N, HH, WW)


# revision 20
# speedup vs baseline: 1.1688x; 1.1688x over previous
"""Trainium2 Bass kernel for nn_AtnScore (masked normalized-correlation softmax).

Math (per batch b):
  w = x2[b] viewed [C, N] (N = H*W, row-major), gram = w^T @ w  [N, N]
  a_l = 10 * (mask_l == 0) / max(||w[:,l]||, 1e-4)
  z[l, n] = a_l * gram[l, n]        (softmax over l, per column n)
  out[l, n] = max(softmax_l(z)[l, n] * (mask_l == 0), 1e-8)

Sharding: 8 cores = 4 batches x 2 column-halves (n in [0,2048) / [2048,4096)).
Each core computes z TRANSPOSED (partition = n-tile of its half, free = l) so
the softmax reduction runs along the free axis; the host gather transposes
back while upcasting.

Masked l columns have z identically 0 (a_l = 0): their softmax weight is
e^0 against a column max of >= 29.6, i.e. < 3e-10 of the sum, and their
outputs clamp to 1e-8. So the HOST packs only the unmasked l columns
(2020..2092 of 4096, padded with zero columns to NU=2176) — halving the
matmul, exp, normalize, and output-DMA work — and scatters the device
result into a 1e-8-filled canvas.

No max-reduce: the exp bias is a host-computed rigorous Cauchy-Schwarz
bound U0(n) = ||x16_n|| * max_l ||a_l x16_l|| boosted by +79; with E in
fp32 the whole column (worst observed slack 141 nats) fits fp32's ~175-nat
range. exp overflow is impossible by construction.

fp16 matmul operands run the PE at full rate (validated 3.2e-3 absmax
error vs the fp32 reference); the output is fp16.
"""

import numpy as np

B, C, HH, WW = 4, 256, 64, 64
N = HH * WW          # 4096 (l dimension, also total n)
NHALF = N // 2       # 2048 columns per core
P = 128              # partitions
KO = C // P          # 2 contraction tiles
NU = 2112            # packed unmasked-l capacity
CB = NU // 2         # 1056 per z tile (3 PSUM banks incl padding)
NT = NHALF // P      # 16 n-tiles per core
BOOST = 79.0

_CACHE = {}


def _build():
    import concourse.bacc as bacc
    import concourse.bass as bass
    import concourse.tile as tile
    import concourse.mybir as mybir
    from concourse.bass import ds

    f32 = mybir.dt.float32
    f16 = mybir.dt.float16
    Alu = mybir.AluOpType
    Act = mybir.ActivationFunctionType

    nc = bacc.Bacc(None, target_bir_lowering=False)

    x2s_d = nc.dram_tensor("x2s16", [C, NU], f16, kind="ExternalInput")
    x2n_d = nc.dram_tensor("x2n16", [C, NHALF], f16, kind="ExternalInput")
    nb_d = nc.dram_tensor("nbias", [P, NT], f32, kind="ExternalInput")
    out_d = nc.dram_tensor("out", [NHALF, NU], f16, kind="ExternalOutput")

    with tile.TileContext(nc) as tc:
        with tc.tile_pool(name="persist", bufs=1) as persist:
            x16s = persist.tile([P, KO, NU], f16)      # moving operand (packed)
            x16n = persist.tile([P, KO, NHALF], f16)   # stationary operand
            nbias = persist.tile([P, NT], f32)
            nc.sync.dma_start(
                x16s[:], x2s_d[:].rearrange("(ko p) n -> p ko n", p=P))
            nc.sync.dma_start(
                x16n[:], x2n_d[:].rearrange("(ko p) n -> p ko n", p=P))
            nc.sync.dma_start(nbias[:], nb_d[:])

            with tc.tile_pool(name="zps", bufs=2, space="PSUM") as zps, \
                 tc.tile_pool(name="ebuf", bufs=3) as ebuf, \
                 tc.tile_pool(name="obuf", bufs=3) as obuf, \
                 tc.tile_pool(name="small", bufs=4) as small:
                # each z tile is CB=1088 wide: matmul sub-chunks 512+512+64
                subs = [(0, 512), (512, 512), (1024, 32)]
                for nt in range(NT):
                    E = ebuf.tile([P, NU], f32, name=f"E{nt}", tag="E")
                    ssum = small.tile([P, 2], f32, name=f"ssum{nt}", tag="ssum")
                    for zt in range(2):
                        z = zps.tile([P, CB], f32, name=f"z{nt}_{zt}", tag="z")
                        for ko in range(KO):
                            for off, w in subs:
                                nc.tensor.matmul(
                                    z[:, ds(off, w)],
                                    x16n[:, ko, ds(nt * P, P)],
                                    x16s[:, ko, ds(zt * CB + off, w)],
                                    start=(ko == 0), stop=(ko == KO - 1))
                        nc.scalar.activation(
                            E[:, ds(zt * CB, CB)], z[:], Act.Exp,
                            bias=nbias[:, ds(nt, 1)], scale=1.0,
                            accum_out=ssum[:, ds(zt, 1)])

                    stot = small.tile([P, 1], f32, name=f"st{nt}", tag="st")
                    nc.vector.reduce_sum(
                        stot[:], ssum[:], axis=mybir.AxisListType.X)
                    nc.vector.tensor_scalar_max(stot[:], stot[:], 1e-30)
                    rtot = small.tile([P, 1], f32, name=f"rt{nt}", tag="rt")
                    nc.vector.reciprocal_approx_fast(rtot[:], stot[:])

                    # normalize into the fp16 output staging tile, DMA out
                    o16 = obuf.tile([P, NU], f16, name=f"o{nt}", tag="o")
                    for zt in range(2):
                        nc.vector.tensor_scalar_mul(
                            o16[:, ds(zt * CB, CB)], E[:, ds(zt * CB, CB)],
                            rtot[:])
                    nc.gpsimd.dma_start(out_d[ds(nt * P, P), :], o16[:])
    nc.finalize()
    return nc


def _get_nc():
    if "nc" not in _CACHE:
        _CACHE["nc"] = _build()
    return _CACHE["nc"]


def _ensure_ntff_hook():
    """bass_utils under axon imports antenv.axon_hooks for trace=True; this
    image's antenv lacks it. Install a stub wired to the boot ctypes hook."""
    import sys
    import types
    try:
        import antenv.axon_hooks  # noqa: F401
        return
    except ImportError:
        pass
    mod = types.ModuleType("antenv.axon_hooks")
    _h = [None]
    mod.set_axon_ntff_profile_hook = lambda hook: _h.__setitem__(0, hook)
    mod.get_axon_ntff_profile_hook = lambda: _h[0]
    sys.modules["antenv.axon_hooks"] = mod
    try:
        import antenv
        antenv.axon_hooks = mod
    except ImportError:
        pass
    try:
        from trn_agent_boot.trn_boot import _ntff_profile_via_ctypes
        hook = _ntff_profile_via_ctypes("/opt/axon/libaxon_pjrt.so")
        if hook is not None:
            mod.set_axon_ntff_profile_hook(hook)
    except Exception:
        pass


def kernel(x2: np.ndarray, mask: np.ndarray) -> np.ndarray:
    from concourse.bass_utils import run_bass_kernel_spmd
    import os

    nc = _get_nc()
    x2 = np.ascontiguousarray(x2, dtype=np.float32)
    mask = np.ascontiguousarray(mask, dtype=np.float32)

    in_maps = []
    idxs = []
    for core in range(8):
        b, h = core // 2, core % 2
        xb = x2[b].reshape(C, N)
        mb = mask[b].reshape(N)
        idx = np.flatnonzero(mb == 0.0)
        assert len(idx) <= NU, f"unmasked count {len(idx)} exceeds NU={NU}"
        idxs.append(idx)
        sumsq = np.einsum("cn,cn->n", xb, xb, dtype=np.float64)
        norm = np.sqrt(sumsq).astype(np.float32)
        a = (10.0 / np.maximum(norm, 1e-4)).astype(np.float32)
        x2s16 = np.zeros((C, NU), dtype=np.float16)
        x2s16[:, :len(idx)] = (xb[:, idx] * a[None, idx]).astype(np.float16)
        x2n16 = np.ascontiguousarray(
            xb[:, h * NHALF:(h + 1) * NHALF]).astype(np.float16)
        # rigorous C-S bound on the f16 dot products, as the exp bias
        n16 = np.linalg.norm(x2n16.astype(np.float32), axis=0)
        y16max = float(np.linalg.norm(x2s16.astype(np.float32), axis=0).max())
        u0 = n16 * y16max * 1.001 + 0.5
        nbias = (BOOST - u0).astype(np.float32).reshape(NT, P).T  # [P, NT]
        in_maps.append({
            "x2s16": x2s16,
            "x2n16": x2n16,
            "nbias": np.ascontiguousarray(nbias),
        })

    trace = bool(int(os.environ.get("ATN_TRACE", "0")))
    if trace:
        _ensure_ntff_hook()
    res = run_bass_kernel_spmd(nc, in_maps, list(range(8)), trace=trace)
    if trace and res.exec_time_ns is not None:
        print(f"HW exec time: {res.exec_time_ns} ns")
        _CACHE["last_exec_ns"] = res.exec_time_ns
        _CACHE["last_results"] = res

    out = np.full((B, N, N), 1e-8, dtype=np.float32)
    for core in range(8):
        b, h = core // 2, core % 2
        idx = idxs[core]
        dev = res.results[core]["out"][:, :len(idx)].astype(np.float32).T
        np.maximum(dev, 1e-8, out=dev)
        out[b][idx, h * NHALF:(h + 1) * NHALF] = dev
    return out.reshape(B, N, HH, WW)
